# revision 4
# baseline (speedup 1.0000x reference)
"""Self-contained Trainium2 Bass kernel for nn_MoEWithDeepEP (8 NeuronCores).

Expert-parallel MoE, DeepEP-style split:
  host   - exact fp32 router (sigmoid top-2 + normalize), dispatch planning,
           token packing (the all-to-all bookkeeping), weighted combine.
  device - ONE fused kernel per core: 8 local experts' SwiGLU GEMMs over
           exactly-counted token segments (tokens ride the matmul free dim,
           so there is no capacity padding in compute) + the shared expert
           over this core's 1/8 token shard.

The per-slot segment capacities depend on the routing realized by the
inputs, so the device program is built (and cached) per capacity tuple.
"""
import sys
for _p in ("/opt/trn_rl_repo", "/root/.axon_site/_ro/trn_rl_repo"):
    if _p not in sys.path:
        sys.path.insert(0, _p)

import numpy as np

N = 8192          # tokens
D = 512           # model dim
E = 64            # experts
K = 2             # top-k
H = 256           # expert hidden
HS = 512          # shared hidden (H * num_shared)
NCORES = 8
NSLOT = E // NCORES   # 8 expert slots per core
NS = N // NCORES      # shared-expert tokens per core
GRP = 256             # shared-expert token group
NGRP = NS // GRP
ROUTE_SCALE = 2.5


def _mk_bacc():
    from concourse import bacc

    return bacc.Bacc(
        "TRN2",
        target_bir_lowering=False,
        debug=False,
        enable_asserts=False,
        num_devices=NCORES,
    )


# ---------------- host-side routing / planning ----------------

def route_and_plan(x, gate_w):
    """Exact fp32 router + expert->core assignment + slot capacities."""
    xf = np.ascontiguousarray(np.asarray(x, np.float32).reshape(N, D))
    logits = xf @ np.asarray(gate_w, np.float32).T
    scores = 1.0 / (1.0 + np.exp(-logits))
    top_idx = np.argsort(-scores, axis=1, kind="stable")[:, :K]
    tsc = np.take_along_axis(scores, top_idx, 1)
    gat = (tsc / (tsc.sum(1, keepdims=True) + 1e-20) * ROUTE_SCALE).astype(
        np.float32
    )
    counts = np.bincount(top_idx.ravel(), minlength=E)

    toks, gvals = [], []
    for e in range(E):
        t, kk = np.nonzero(top_idx == e)
        toks.append(t)
        gvals.append(gat[t, kk])

    # LPT: exactly NSLOT experts per core, balancing token load
    order = np.argsort(-counts, kind="stable")
    loads = [0] * NCORES
    assign = [[] for _ in range(NCORES)]
    for e in order:
        c = min(
            (c for c in range(NCORES) if len(assign[c]) < NSLOT),
            key=lambda c: loads[c],
        )
        assign[c].append(int(e))
        loads[c] += int(counts[e])
    for c in range(NCORES):
        assign[c].sort(key=lambda e: -counts[e])

    # slot capacities: rank-wise max across cores, 8-aligned
    caps = tuple(
        int(-(-max(counts[assign[c] [j]] for c in range(NCORES)) // 8) * 8)
        if max(counts[assign[c][j]] for c in range(NCORES)) > 0 else 8
        for j in range(NSLOT)
    )
    return xf, toks, gvals, assign, caps


# ---------------- device kernel ----------------

def build_kernel(caps):
    import concourse.bass as bass
    import concourse.tile as tile
    from concourse import mybir

    dt = mybir.dt
    AF = mybir.ActivationFunctionType
    OP = mybir.AluOpType
    ts = bass.ts
    nc = _mk_bacc()

    starts = np.concatenate([[0], np.cumsum(caps)]).astype(int)
    P = int(starts[-1])

    xd = nc.dram_tensor("xd", [D, P], dt.float16, kind="ExternalInput")
    w1s = nc.dram_tensor("w1s", [NSLOT, D, H], dt.float16, kind="ExternalInput")
    w3s = nc.dram_tensor("w3s", [NSLOT, D, H], dt.float16, kind="ExternalInput")
    w2s = nc.dram_tensor("w2s", [NSLOT, H, D], dt.float16, kind="ExternalInput")
    sw1 = nc.dram_tensor("sw1", [D, HS], dt.float16, kind="ExternalInput")
    sw3 = nc.dram_tensor("sw3", [D, HS], dt.float16, kind="ExternalInput")
    sw2 = nc.dram_tensor("sw2", [HS, D], dt.float16, kind="ExternalInput")
    xsT = nc.dram_tensor("xsT", [D, NS], dt.float16, kind="ExternalInput")

    yT = nc.dram_tensor("yT", [D, P], dt.float16, kind="ExternalOutput")
    ysh = nc.dram_tensor("ysh", [NS, D], dt.float16, kind="ExternalOutput")

    with tile.TileContext(nc) as tc:
        with (
            tc.tile_pool(name="const", bufs=1) as cpool,
            tc.tile_pool(name="ew", bufs=3) as ewpool,
            tc.tile_pool(name="xe", bufs=3) as xpool,
            tc.tile_pool(name="work", bufs=2) as wpool,
            tc.tile_pool(name="hps", bufs=4, space="PSUM") as hpsum,
            tc.tile_pool(name="yps", bufs=2, space="PSUM") as ypsum,
        ):
            sw1_sb = cpool.tile([128, 4, HS], dt.float16, name="sw1_sb")
            sw3_sb = cpool.tile([128, 4, HS], dt.float16, name="sw3_sb")
            sw2_sb = cpool.tile([128, 4, D], dt.float16, name="sw2_sb")
            xs_sb = cpool.tile([128, 4, NS], dt.float16, name="xs_sb")

            def load_consts():
                nc.sync.dma_start(
                    sw1_sb[:], sw1.ap().rearrange("(c p) h -> p c h", p=128)
                )
                nc.sync.dma_start(
                    sw3_sb[:], sw3.ap().rearrange("(c p) h -> p c h", p=128)
                )
                nc.sync.dma_start(
                    sw2_sb[:], sw2.ap().rearrange("(c p) d -> p c d", p=128)
                )

            def load_xs(g):
                nc.sync.dma_start(
                    xs_sb[:, :, ts(g, GRP)],
                    xsT.ap()[:, ts(g, GRP)].rearrange("(c p) t -> p c t", p=128),
                )

            def expert(j):
                C = int(caps[j])
                s0 = int(starts[j])
                w1_sb = ewpool.tile([128, 4, H], dt.float16, tag="w1", name="w1_sb")
                nc.sync.dma_start(
                    w1_sb[:], w1s.ap()[j].rearrange("(c p) h -> p c h", p=128)
                )
                w3_sb = ewpool.tile([128, 4, H], dt.float16, tag="w3", name="w3_sb")
                nc.sync.dma_start(
                    w3_sb[:], w3s.ap()[j].rearrange("(c p) h -> p c h", p=128)
                )
                w2_sb = ewpool.tile([128, 2, D], dt.float16, tag="w2", name="w2_sb")
                nc.sync.dma_start(
                    w2_sb[:], w2s.ap()[j].rearrange("(c p) d -> p c d", p=128)
                )
                xe = xpool.tile([128, 4, C], dt.float16, tag="xe", name="xe")
                nc.sync.dma_start(
                    xe[:],
                    xd.ap()[:, s0:s0 + C].rearrange("(c p) t -> p c t", p=128),
                )

                he = wpool.tile([128, 2, C], dt.float16, tag="he", name="he")
                for hc in range(2):
                    ph1 = hpsum.tile([128, C], dt.float32, tag="ph", name="ph1")
                    for c in range(4):
                        nc.tensor.matmul(
                            ph1[:], lhsT=w1_sb[:, c, ts(hc, 128)], rhs=xe[:, c, :],
                            start=(c == 0), stop=(c == 3),
                        )
                    ph3 = hpsum.tile([128, C], dt.float32, tag="ph", name="ph3")
                    for c in range(4):
                        nc.tensor.matmul(
                            ph3[:], lhsT=w3_sb[:, c, ts(hc, 128)], rhs=xe[:, c, :],
                            start=(c == 0), stop=(c == 3),
                        )
                    t1 = wpool.tile([128, C], dt.float32, tag="t1", name="t1")
                    nc.scalar.activation(t1[:], ph1[:], AF.Silu)
                    nc.vector.tensor_tensor(
                        out=he[:, hc, :], in0=t1[:], in1=ph3[:], op=OP.mult
                    )

                yb = wpool.tile([128, 4, C], dt.float16, tag="yb", name="yb")
                for dc in range(4):
                    py = ypsum.tile([128, C], dt.float32, tag="py", name="py")
                    for hc in range(2):
                        nc.tensor.matmul(
                            py[:], lhsT=w2_sb[:, hc, ts(dc, 128)], rhs=he[:, hc, :],
                            start=(hc == 0), stop=(hc == 1),
                        )
                    nc.scalar.copy(yb[:, dc, :], py[:])
                nc.sync.dma_start(
                    yT.ap()[:, s0:s0 + C].rearrange("(c p) t -> p c t", p=128),
                    yb[:],
                )

            def shared_group(g):
                hsh = wpool.tile([128, 4, GRP], dt.float16, tag="hsh", name="hsh")
                for hc in range(4):
                    ph1 = hpsum.tile([128, GRP], dt.float32, tag="ph", name="sph1")
                    for c in range(4):
                        nc.tensor.matmul(
                            ph1[:], lhsT=sw1_sb[:, c, ts(hc, 128)],
                            rhs=xs_sb[:, c, ts(g, GRP)],
                            start=(c == 0), stop=(c == 3),
                        )
                    ph3 = hpsum.tile([128, GRP], dt.float32, tag="ph", name="sph3")
                    for c in range(4):
                        nc.tensor.matmul(
                            ph3[:], lhsT=sw3_sb[:, c, ts(hc, 128)],
                            rhs=xs_sb[:, c, ts(g, GRP)],
                            start=(c == 0), stop=(c == 3),
                        )
                    t1 = wpool.tile([128, GRP], dt.float32, tag="t1", name="st1")
                    nc.scalar.activation(t1[:], ph1[:], AF.Silu)
                    nc.vector.tensor_tensor(
                        out=hsh[:, hc, :], in0=t1[:], in1=ph3[:], op=OP.mult
                    )
                yg = wpool.tile([128, GRP // 128, D], dt.float16, tag="yg", name="yg")
                for t2 in range(GRP // 128):
                    py = ypsum.tile([128, D], dt.float32, tag="py", name="spy")
                    for hc in range(4):
                        nc.tensor.matmul(
                            py[:], lhsT=hsh[:, hc, ts(t2, 128)],
                            rhs=sw2_sb[:, hc, :],
                            start=(hc == 0), stop=(hc == 3),
                        )
                    nc.vector.tensor_copy(yg[:, t2, :], py[:])
                nc.sync.dma_start(
                    ysh.ap()[ts(g, GRP), :].rearrange("(t p) d -> p t d", p=128),
                    yg[:],
                )

            expert(0)
            expert(1)
            load_consts()
            for g in range(NGRP):
                load_xs(g)
            shared_group(0)
            expert(2)
            expert(3)
            shared_group(1)
            expert(4)
            expert(5)
            shared_group(2)
            expert(6)
            expert(7)
            shared_group(3)

    nc.compile()
    return nc


# ---------------- host-side pack / combine ----------------

def host_prepare(xf, toks, assign, caps, w1, w3, w2, sw1, sw3, sw2):
    starts = np.concatenate([[0], np.cumsum(caps)]).astype(int)
    P = int(starts[-1])
    xfT16 = np.ascontiguousarray(xf.T.astype(np.float16))  # [D, N]
    w1h = np.asarray(w1, np.float32).astype(np.float16)
    w3h = np.asarray(w3, np.float32).astype(np.float16)
    w2h = np.asarray(w2, np.float32).astype(np.float16)
    sw1h = np.ascontiguousarray(np.asarray(sw1, np.float32).astype(np.float16))
    sw3h = np.ascontiguousarray(np.asarray(sw3, np.float32).astype(np.float16))
    sw2h = np.ascontiguousarray(np.asarray(sw2, np.float32).astype(np.float16))
    in_maps = []
    for c in range(NCORES):
        cols = np.zeros(P, np.int64)
        for j, e in enumerate(assign[c]):
            t = toks[e]
            cols[starts[j]:starts[j] + len(t)] = t
        in_maps.append({
            "xd": np.ascontiguousarray(xfT16[:, cols]),
            "w1s": np.ascontiguousarray(w1h[assign[c]]),
            "w3s": np.ascontiguousarray(w3h[assign[c]]),
            "w2s": np.ascontiguousarray(w2h[assign[c]]),
            "sw1": sw1h,
            "sw3": sw3h,
            "sw2": sw2h,
            "xsT": np.ascontiguousarray(xfT16[:, c * NS:(c + 1) * NS]),
        })
    return in_maps, starts


def host_combine(res, toks, gvals, assign, starts):
    out = np.zeros((N, D), np.float32)
    for c, r in enumerate(res):
        yTc = np.asarray(r["yT"], np.float32)   # [D, P]
        for j, e in enumerate(assign[c]):
            t = toks[e]
            n = len(t)
            if n == 0:
                continue
            s0 = int(starts[j])
            # tokens within one expert are unique -> fancy-index add is safe
            out[t] += yTc[:, s0:s0 + n].T * gvals[e][:, None]
        out[c * NS:(c + 1) * NS] += np.asarray(r["ysh"], np.float32)
    return out.reshape(4, 2048, D)


_CACHE = {}


def kernel(x, gate_w, w1, w3, w2, sw1, sw3, sw2):
    from concourse.bass_utils import run_bass_kernel_spmd

    xf, toks, gvals, assign, caps = route_and_plan(x, gate_w)
    if caps not in _CACHE:
        _CACHE[caps] = build_kernel(caps)
    nc = _CACHE[caps]

    in_maps, starts = host_prepare(
        xf, toks, assign, caps, w1, w3, w2, sw1, sw3, sw2
    )
    res = run_bass_kernel_spmd(nc, in_maps, core_ids=list(range(NCORES))).results
    return host_combine(res, toks, gvals, assign, starts).astype(np.float32)


# revision 6
# speedup vs baseline: 1.0302x; 1.0302x over previous
"""Self-contained Trainium2 Bass kernel for nn_MoEWithDeepEP (8 NeuronCores).

Expert-parallel MoE, DeepEP-style split:
  host   - exact fp32 router (sigmoid top-2 + normalize), dispatch planning,
           token packing (the all-to-all bookkeeping), weighted combine.
  device - ONE fused kernel per core: 8 local experts' SwiGLU GEMMs over
           exactly-counted token segments (tokens ride the matmul free dim,
           so there is no capacity padding in compute) + the shared expert
           over this core's 1/8 token shard.

All device tensors are pre-shuffled on the host into partition-major
[128, F] contiguous layouts so every DMA is a dense 2D copy (few, large
descriptors). DMA issue is spread over four engine queues. The PE is
warmed with throwaway matmuls during the input DMA lead-in so real work
runs at full clock.

The per-slot segment capacities depend on the routing realized by the
inputs, so the device program is built (and cached) per capacity tuple.
"""
import sys
for _p in ("/opt/trn_rl_repo", "/root/.axon_site/_ro/trn_rl_repo"):
    if _p not in sys.path:
        sys.path.insert(0, _p)

import numpy as np

N = 8192          # tokens
D = 512           # model dim
E = 64            # experts
K = 2             # top-k
H = 256           # expert hidden
HS = 512          # shared hidden (H * num_shared)
NCORES = 8
NSLOT = E // NCORES   # 8 expert slots per core
NS = N // NCORES      # shared-expert tokens per core
GRP = 256             # shared-expert token group
NGRP = NS // GRP
ROUTE_SCALE = 2.5
WARM_MM = 10          # PE warmup matmuls during DMA lead-in


def _mk_bacc():
    from concourse import bacc

    return bacc.Bacc(
        "TRN2",
        target_bir_lowering=False,
        debug=False,
        enable_asserts=False,
        num_devices=NCORES,
    )


# ---------------- host-side routing / planning ----------------

def route_and_plan(x, gate_w):
    """Exact fp32 router + expert->core assignment + slot capacities."""
    xf = np.ascontiguousarray(np.asarray(x, np.float32).reshape(N, D))
    logits = xf @ np.asarray(gate_w, np.float32).T
    scores = 1.0 / (1.0 + np.exp(-logits))
    top_idx = np.argsort(-scores, axis=1, kind="stable")[:, :K]
    tsc = np.take_along_axis(scores, top_idx, 1)
    gat = (tsc / (tsc.sum(1, keepdims=True) + 1e-20) * ROUTE_SCALE).astype(
        np.float32
    )
    counts = np.bincount(top_idx.ravel(), minlength=E)

    toks, gvals = [], []
    for e in range(E):
        t, kk = np.nonzero(top_idx == e)
        toks.append(t)
        gvals.append(gat[t, kk])

    # LPT: exactly NSLOT experts per core, balancing token load
    order = np.argsort(-counts, kind="stable")
    loads = [0] * NCORES
    assign = [[] for _ in range(NCORES)]
    for e in order:
        c = min(
            (c for c in range(NCORES) if len(assign[c]) < NSLOT),
            key=lambda c: loads[c],
        )
        assign[c].append(int(e))
        loads[c] += int(counts[e])
    for c in range(NCORES):
        assign[c].sort(key=lambda e: -counts[e])

    # slot capacities: rank-wise max across cores, 8-aligned
    caps = tuple(
        max(8, int(-(-max(counts[assign[c][j]] for c in range(NCORES)) // 8) * 8))
        for j in range(NSLOT)
    )
    return xf, toks, gvals, assign, caps


# ---------------- device kernel ----------------

def build_kernel(caps):
    import concourse.bass as bass
    import concourse.tile as tile
    from concourse import mybir

    dt = mybir.dt
    AF = mybir.ActivationFunctionType
    OP = mybir.AluOpType
    ts = bass.ts
    nc = _mk_bacc()

    starts = np.concatenate([[0], np.cumsum(caps)]).astype(int)
    P = int(starts[-1])

    # all inputs/outputs pre-shuffled to partition-major [128, F] layouts
    xd = nc.dram_tensor("xd", [128, 4 * P], dt.float16, kind="ExternalInput")
    w13s = nc.dram_tensor("w13s", [NSLOT, 128, 8 * H], dt.float16,
                          kind="ExternalInput")
    w2s = nc.dram_tensor("w2s", [NSLOT, 128, 2 * D], dt.float16,
                         kind="ExternalInput")
    sw13 = nc.dram_tensor("sw13", [128, 8 * HS], dt.float16, kind="ExternalInput")
    sw2p = nc.dram_tensor("sw2p", [128, 4 * D], dt.float16, kind="ExternalInput")
    xsp = nc.dram_tensor("xsp", [128, NGRP * 4 * GRP], dt.float16,
                         kind="ExternalInput")

    yT = nc.dram_tensor("yT", [128, 4 * P], dt.float16, kind="ExternalOutput")
    ysh = nc.dram_tensor("ysh", [128, NGRP * 2 * D], dt.float16,
                         kind="ExternalOutput")

    with tile.TileContext(nc) as tc:
        with (
            tc.tile_pool(name="const", bufs=1) as cpool,
            tc.tile_pool(name="ew", bufs=3) as ewpool,
            tc.tile_pool(name="xe", bufs=3) as xpool,
            tc.tile_pool(name="work", bufs=2) as wpool,
            tc.tile_pool(name="hps", bufs=4, space="PSUM") as hpsum,
            tc.tile_pool(name="yps", bufs=2, space="PSUM") as ypsum,
            tc.tile_pool(name="warm", bufs=1, space="PSUM") as warmpool,
        ):
            # ---- PE warmup: throwaway matmuls with no input deps ----
            wsrc = cpool.tile([128, 128], dt.float16, name="wsrc")
            nc.vector.memset(wsrc[:], 0)
            wps = warmpool.tile([128, 512], dt.float32, tag="wps", name="wps")
            for _ in range(WARM_MM):
                nc.tensor.matmul(
                    wps[:], lhsT=wsrc[:], rhs=wsrc[:, 0:1].to_broadcast([128, 512]),
                    start=True, stop=True, skip_group_check=True,
                )

            sw13_sb = cpool.tile([128, 8, HS], dt.float16, name="sw13_sb")
            sw2_sb = cpool.tile([128, 4, D], dt.float16, name="sw2_sb")
            xs_sb = cpool.tile([128, NGRP, 4, GRP], dt.float16, name="xs_sb")

            def load_consts():
                nc.sync.dma_start(
                    sw13_sb[:], sw13.ap().rearrange("p (c h) -> p c h", c=8)
                )
                nc.scalar.dma_start(
                    sw2_sb[:], sw2p.ap().rearrange("p (c d) -> p c d", c=4)
                )

            def load_xs(g):
                nc.gpsimd.dma_start(
                    xs_sb[:, g],
                    xsp.ap()[:, ts(g, 4 * GRP)].rearrange(
                        "p (c t) -> p c t", c=4
                    ),
                )

            def expert(j):
                C = int(caps[j])
                s0 = int(starts[j])
                w13_sb = ewpool.tile([128, 8, H], dt.float16, tag="w13",
                                     name="w13_sb")
                nc.sync.dma_start(
                    w13_sb[:], w13s.ap()[j].rearrange("p (c h) -> p c h", c=8)
                )
                xe = xpool.tile([128, 4, C], dt.float16, tag="xe", name="xe")
                nc.gpsimd.dma_start(
                    xe[:],
                    xd.ap()[:, 4 * s0:4 * s0 + 4 * C].rearrange(
                        "p (c t) -> p c t", c=4
                    ),
                )
                w2_sb = ewpool.tile([128, 2, D], dt.float16, tag="w2",
                                    name="w2_sb")
                nc.scalar.dma_start(
                    w2_sb[:], w2s.ap()[j].rearrange("p (c d) -> p c d", c=2)
                )

                he = wpool.tile([128, 2, C], dt.float16, tag="he", name="he")
                for hc in range(2):
                    ph1 = hpsum.tile([128, C], dt.float32, tag="ph", name="ph1")
                    for c in range(4):
                        nc.tensor.matmul(
                            ph1[:], lhsT=w13_sb[:, c, ts(hc, 128)],
                            rhs=xe[:, c, :], start=(c == 0), stop=(c == 3),
                        )
                    ph3 = hpsum.tile([128, C], dt.float32, tag="ph", name="ph3")
                    for c in range(4):
                        nc.tensor.matmul(
                            ph3[:], lhsT=w13_sb[:, 4 + c, ts(hc, 128)],
                            rhs=xe[:, c, :], start=(c == 0), stop=(c == 3),
                        )
                    t1 = wpool.tile([128, C], dt.float32, tag="t1", name="t1")
                    nc.scalar.activation(t1[:], ph1[:], AF.Silu)
                    nc.vector.tensor_tensor(
                        out=he[:, hc, :], in0=t1[:], in1=ph3[:], op=OP.mult
                    )

                yb = wpool.tile([128, 4, C], dt.float16, tag="yb", name="yb")
                for dc in range(4):
                    py = ypsum.tile([128, C], dt.float32, tag="py", name="py")
                    for hc in range(2):
                        nc.tensor.matmul(
                            py[:], lhsT=w2_sb[:, hc, ts(dc, 128)],
                            rhs=he[:, hc, :], start=(hc == 0), stop=(hc == 1),
                        )
                    nc.scalar.copy(yb[:, dc, :], py[:])
                nc.sync.dma_start(
                    yT.ap()[:, 4 * s0:4 * s0 + 4 * C].rearrange(
                        "p (c t) -> p c t", c=4
                    ),
                    yb[:],
                )

            def shared_group(g):
                hsh = wpool.tile([128, 4, GRP], dt.float16, tag="hsh", name="hsh")
                for hc in range(4):
                    ph1 = hpsum.tile([128, GRP], dt.float32, tag="ph", name="sph1")
                    for c in range(4):
                        nc.tensor.matmul(
                            ph1[:], lhsT=sw13_sb[:, c, ts(hc, 128)],
                            rhs=xs_sb[:, g, c, :], start=(c == 0), stop=(c == 3),
                        )
                    ph3 = hpsum.tile([128, GRP], dt.float32, tag="ph", name="sph3")
                    for c in range(4):
                        nc.tensor.matmul(
                            ph3[:], lhsT=sw13_sb[:, 4 + c, ts(hc, 128)],
                            rhs=xs_sb[:, g, c, :], start=(c == 0), stop=(c == 3),
                        )
                    t1 = wpool.tile([128, GRP], dt.float32, tag="t1", name="st1")
                    nc.scalar.activation(t1[:], ph1[:], AF.Silu)
                    nc.vector.tensor_tensor(
                        out=hsh[:, hc, :], in0=t1[:], in1=ph3[:], op=OP.mult
                    )
                yg = wpool.tile([128, 2, D], dt.float16, tag="yg", name="yg")
                for t2 in range(2):
                    py = ypsum.tile([128, D], dt.float32, tag="py", name="spy")
                    for hc in range(4):
                        nc.tensor.matmul(
                            py[:], lhsT=hsh[:, hc, ts(t2, 128)],
                            rhs=sw2_sb[:, hc, :], start=(hc == 0), stop=(hc == 3),
                        )
                    nc.vector.tensor_copy(yg[:, t2, :], py[:])
                nc.gpsimd.dma_start(
                    ysh.ap()[:, ts(g, 2 * D)].rearrange("p (c d) -> p c d", c=2),
                    yg[:],
                )

            # smallest experts first (fastest time-to-first-matmul)
            sched = [7, 6, "c", "g0", 5, 4, "g1", 3, 2, "g2", 1, 0, "g3"]
            for step in sched:
                if step == "c":
                    load_consts()
                    for g in range(NGRP):
                        load_xs(g)
                elif isinstance(step, str) and step.startswith("g"):
                    shared_group(int(step[1:]))
                else:
                    expert(step)

    nc.compile()
    return nc


# ---------------- host-side pack / combine ----------------

def _pshuf(a, nchunk):
    """[nchunk*128, F] -> [128, nchunk*F] partition-major contiguous."""
    f = a.shape[-1]
    return np.ascontiguousarray(
        a.reshape(nchunk, 128, f).transpose(1, 0, 2).reshape(128, nchunk * f)
    )


def host_prepare(xf, toks, assign, caps, w1, w3, w2, sw1, sw3, sw2):
    starts = np.concatenate([[0], np.cumsum(caps)]).astype(int)
    P = int(starts[-1])
    xfT16 = xf.T.astype(np.float16)                    # [D, N]
    x4 = xfT16.reshape(4, 128, N).transpose(1, 0, 2)   # [128, 4, N]
    w1h = np.asarray(w1, np.float32).astype(np.float16)
    w3h = np.asarray(w3, np.float32).astype(np.float16)
    w2h = np.asarray(w2, np.float32).astype(np.float16)
    # per-expert partition-major slabs
    w13p = np.empty((E, 128, 8 * H), np.float16)
    w13p[:, :, :4 * H] = (
        w1h.reshape(E, 4, 128, H).transpose(0, 2, 1, 3).reshape(E, 128, 4 * H)
    )
    w13p[:, :, 4 * H:] = (
        w3h.reshape(E, 4, 128, H).transpose(0, 2, 1, 3).reshape(E, 128, 4 * H)
    )
    w2p = w2h.reshape(E, 2, 128, D).transpose(0, 2, 1, 3).reshape(E, 128, 2 * D)
    sw13h = np.empty((128, 8 * HS), np.float16)
    sw13h[:, :4 * HS] = _pshuf(np.asarray(sw1, np.float32).astype(np.float16), 4)
    sw13h[:, 4 * HS:] = _pshuf(np.asarray(sw3, np.float32).astype(np.float16), 4)
    sw2ph = _pshuf(np.asarray(sw2, np.float32).astype(np.float16), 4)

    in_maps = []
    for c in range(NCORES):
        cols = np.zeros(P, np.int64)
        for j, e in enumerate(assign[c]):
            t = toks[e]
            cols[starts[j]:starts[j] + len(t)] = t
        xdc = x4[:, :, cols]                            # [128, 4, P]
        # segment-major flatten: expert j occupies cols [4*s_j, 4*s_j+4*C_j)
        xdp = np.empty((128, 4 * P), np.float16)
        for j in range(NSLOT):
            s0, C = int(starts[j]), int(caps[j])
            xdp[:, 4 * s0:4 * s0 + 4 * C] = xdc[:, :, s0:s0 + C].reshape(128, -1)
        xs = x4[:, :, c * NS:(c + 1) * NS]              # [128, 4, NS]
        xspc = np.empty((128, NGRP * 4 * GRP), np.float16)
        for g in range(NGRP):
            xspc[:, g * 4 * GRP:(g + 1) * 4 * GRP] = (
                xs[:, :, g * GRP:(g + 1) * GRP].reshape(128, -1)
            )
        in_maps.append({
            "xd": np.ascontiguousarray(xdp),
            "w13s": np.ascontiguousarray(w13p[assign[c]]),
            "w2s": np.ascontiguousarray(w2p[assign[c]]),
            "sw13": sw13h,
            "sw2p": sw2ph,
            "xsp": np.ascontiguousarray(xspc),
        })
    return in_maps, starts


def host_combine(res, toks, gvals, assign, starts):
    out = np.zeros((N, D), np.float32)
    for c, r in enumerate(res):
        yTc = np.asarray(r["yT"])                       # [128, 4*P]
        for j, e in enumerate(assign[c]):
            t = toks[e]
            n = len(t)
            if n == 0:
                continue
            s0 = int(starts[j])
            C = int(starts[j + 1]) - s0
            blk = yTc[:, 4 * s0:4 * s0 + 4 * C].reshape(128, 4, C)[:, :, :n]
            yseg = blk.transpose(2, 1, 0).reshape(n, D).astype(np.float32)
            out[t] += yseg * gvals[e][:, None]
        yshc = np.asarray(r["ysh"]).reshape(128, NGRP, 2, D)
        ysh_rows = yshc.transpose(1, 2, 0, 3).reshape(NS, D).astype(np.float32)
        out[c * NS:(c + 1) * NS] += ysh_rows
    return out.reshape(4, 2048, D)


_CACHE = {}


def kernel(x, gate_w, w1, w3, w2, sw1, sw3, sw2):
    from concourse.bass_utils import run_bass_kernel_spmd

    xf, toks, gvals, assign, caps = route_and_plan(x, gate_w)
    if caps not in _CACHE:
        _CACHE[caps] = build_kernel(caps)
    nc = _CACHE[caps]

    in_maps, starts = host_prepare(
        xf, toks, assign, caps, w1, w3, w2, sw1, sw3, sw2
    )
    res = run_bass_kernel_spmd(nc, in_maps, core_ids=list(range(NCORES))).results
    return host_combine(res, toks, gvals, assign, starts).astype(np.float32)
